# revision 15
# baseline (speedup 1.0000x reference)
"""JacobiGNN Trainium2 kernel: out = log_softmax(U @ (H * (U^T z)), axis=1).

Sharding: column-shard U across 8 cores (1024 spectral cols each). Every core
computes the full z = MLP(x) locally (x streamed in chunks; no collective on
the critical path). Per core, U is streamed from DRAM exactly once in bf16
(host-pretiled for contiguous DMA); each 128x128 tile is loaded into the PE
array once as stationary weights, against which we stream [z | I128] (-> G
contribution and the transposed tile in one pass). out^T partials accumulate
in PSUM packed 4x16 partitions over all 8 blocks; a single bf16 ReduceScatter
in [p,j,c] layout sums partials across cores; log_softmax runs on the
(row-interleaved) local shard and the host undoes the interleaving.
"""

import os
import sys

import numpy as np

for _p in ("/opt/trn_rl_repo", "/root/.axon_site/_ro/trn_rl_repo"):
    if os.path.isdir(_p) and _p not in sys.path:
        sys.path.insert(0, _p)

import concourse.bacc as bacc
import concourse.bass as bass  # noqa: F401
import concourse.mybir as mybir
import concourse.tile as tile
from concourse.bass_utils import run_bass_kernel_spmd

F32 = mybir.dt.float32
BF16 = mybir.dt.bfloat16
N, F_IN, HID, C, K = 8192, 512, 64, 16, 10
BASE_ALPHA = 0.5
JA, JB, JL, JR = 1.0, 1.0, -1.0, 1.0
NCORES = 8
SH = N // NCORES      # spectral columns per core (1024)
NB = SH // 128        # column blocks per core (8)
RCH = N // 128        # row chunks (64)
MYR = SH // 128       # local row chunks (8)
XCH = 8               # x/z processed in XCH chunks of N//XCH rows

_CACHE = {}


def _jacobi_coef_rows(temp):
    """Host-precomputed per-channel coefficient rows, [30*C] packed."""
    a, b, l, r = JA, JB, JL, JR
    alphas = (BASE_ALPHA * np.tanh(np.asarray(temp, np.float64)))  # [C, K+1]
    rows = [alphas[:, 0]]
    coef1 = (a - b) / 2 - (a + b + 2) / 2 * (l + r) / (r - l)
    coef2 = (a + b + 2) / (r - l)
    rows.append(coef1 * alphas[:, 1])   # c1_0
    rows.append(coef2 * alphas[:, 1])   # c1_1
    for L in range(2, K + 1):
        coef_l = 2 * L * (L + a + b) * (2 * L - 2 + a + b)
        c_lm1_1 = (2 * L + a + b - 1) * (2 * L + a + b) * (2 * L + a + b - 2)
        c_lm1_2 = (2 * L + a + b - 1) * (a ** 2 - b ** 2)
        c_lm2 = 2 * (L - 1 + a) * (L - 1 + b) * (2 * L + a + b)
        tmp1 = alphas[:, L - 1] * (c_lm1_1 / coef_l)
        tmp2 = alphas[:, L - 1] * (c_lm1_2 / coef_l)
        tmp3 = alphas[:, L - 1] * alphas[:, L - 2] * (c_lm2 / coef_l)
        rows.append(tmp1 * (2 / (r - l)))                    # t1
        rows.append(tmp1 * ((r + l) / (r - l)) + tmp2)       # t2
        rows.append(tmp3)                                    # t3
    packed = np.concatenate(rows).astype(np.float32).reshape(1, 30 * C)
    return np.ascontiguousarray(np.repeat(packed, 128, axis=0))


def _bc(ap, shape, axis=1):
    """Broadcast an AP to a 3D [128, NB, C]-style shape with stride-0 dims."""
    while ap.ndim < len(shape):
        ap = ap.unsqueeze(axis)
    return ap.broadcast_to(shape)


def _build():
    nc = bacc.Bacc("TRN2", target_bir_lowering=False, debug=False)

    # u_prep[b, p, g, j, c] = U[1024g + 128j + p, 1024i + 128b + c] (bf16)
    u_pr = nc.dram_tensor("u_prep", [NB, 128, 8, 8, 128], BF16, kind="ExternalInput")
    x_ft = nc.dram_tensor("x_fullT", [F_IN, N], BF16, kind="ExternalInput")
    e_sh = nc.dram_tensor("e_shard", [128, MYR], F32, kind="ExternalInput")
    w1r = nc.dram_tensor("w1r", [128, 4 * HID], BF16, kind="ExternalInput")
    w2d = nc.dram_tensor("w2d", [HID, C], BF16, kind="ExternalInput")
    b1c = nc.dram_tensor("b1c", [HID, 1], F32, kind="ExternalInput")
    b2r = nc.dram_tensor("b2r", [128, C], F32, kind="ExternalInput")
    jcd = nc.dram_tensor("jcd", [128, 30 * C], F32, kind="ExternalInput")
    id128d = nc.dram_tensor("id128d", [128, 128], BF16, kind="ExternalInput")
    id16x4d = nc.dram_tensor("id16x4d", [128, C], F32, kind="ExternalInput")
    # out_shard flat f = q*1024 + j*16 + c maps to out[128j + 16i + q, c]
    out_sh = nc.dram_tensor("out_shard", [128, MYR * C], F32, kind="ExternalOutput")

    rg = [list(range(NCORES))]
    XR = N // XCH // 128   # row chunks per x chunk (8)

    with nc.allow_low_precision(reason="bf16 matmul path, fp32 accumulation"), \
         tile.TileContext(nc) as tc:
        with (
            tc.tile_pool(name="dram", bufs=1, space="DRAM") as dram,
            tc.tile_pool(name="consts", bufs=1) as cp,
            tc.tile_pool(name="persist", bufs=1) as pp,
            tc.tile_pool(name="xsb", bufs=2) as xp,
            tc.tile_pool(name="usb", bufs=3) as up,
            tc.tile_pool(name="utsb", bufs=2) as utp,
            tc.tile_pool(name="small", bufs=4) as sp,
        ):
            rs_in = dram.tile([128, RCH, C], BF16)
            rs_out = dram.tile([C, RCH, C], BF16)

            id128 = cp.tile_from(id128d[:])
            id16x4 = cp.tile_from(id16x4d[:])
            jc = cp.tile_from(jcd[:])
            w1 = cp.tile_from(w1r[:])
            w2 = cp.tile_from(w2d[:])
            b1 = cp.tile_from(b1c[:])
            b2 = cp.tile_from(b2r[:])
            e_col = cp.tile_from(e_sh[:])

            # ---- persistent SBUF ----
            zid = pp.tile([128, RCH, C + 128], BF16)  # [z_chunk | I128] per row chunk
            h_sb = pp.tile([HID, N], BF16)
            hacc = pp.tile([128, NB, C], F32)        # Jacobi filter H
            xs_a = pp.tile([128, NB, C], F32)
            xs_b = pp.tile([128, NB, C], F32)
            htmp = pp.tile([128, NB, C], F32)
            htmp2 = pp.tile([128, NB, C], F32)
            accsb = pp.tile([128, 2048], F32)        # out^T, packed 4x16
            o_all = pp.tile([128, RCH, C], BF16)     # out rows, [p, j, c]
            smin = pp.tile([128, MYR, C], BF16)      # RS result (local shard)
            mneg = pp.tile([128, MYR, 1], F32)
            sexp = pp.tile([128, MYR, C], F32)       # x - max
            set_ = pp.tile([128, MYR, C], F32)       # exp(x - max)
            ssum = pp.tile([128, MYR], F32)
            lns = pp.tile([128, MYR], F32)
            smout = pp.tile([128, MYR, C], F32)
            # identity halves of zid
            nc.gpsimd.dma_start(
                out=zid[:, :, C:C + 128],
                in_=id128[:].unsqueeze(1).broadcast_to((128, RCH, 128)))

            # ========= phase 0: full z = MLP(x) computed locally =========
            with tc.tile_pool(name="ppre", bufs=1, space="PSUM") as ppre:
                for q in range(XCH):
                    CW = N // XCH  # 1024 columns (rows of x) per chunk
                    xT = xp.tile([128, 4, CW], BF16, tag="xT")
                    nc.scalar.dma_start(
                        out=xT[:],
                        in_=x_ft[:, q * CW:(q + 1) * CW]
                        .rearrange("(a p) r -> p a r", p=128))
                    ph = ppre.tile([HID, CW], F32, tag="ph")
                    for half in range(CW // 512):
                        for fb in range(4):
                            nc.tensor.matmul(
                                ph[:, half * 512:(half + 1) * 512],
                                lhsT=w1[:, fb * HID:(fb + 1) * HID],
                                rhs=xT[:, fb, half * 512:(half + 1) * 512],
                                start=(fb == 0), stop=(fb == 3),
                            )
                    nc.scalar.activation(
                        h_sb[:, q * CW:(q + 1) * CW], ph[:],
                        mybir.ActivationFunctionType.Relu,
                        bias=b1[:, 0:1], scale=1.0)
                    pzq = ppre.tile([128, XR, C], F32, tag="pz", bufs=2)
                    for j in range(XR):
                        rc = q * XR + j
                        nc.tensor.matmul(
                            pzq[:, j, :],
                            lhsT=h_sb[:, rc * 128:(rc + 1) * 128], rhs=w2[:],
                            start=True, stop=True,
                        )
                    nc.vector.tensor_add(
                        zid[:, q * XR:(q + 1) * XR, 0:C], pzq[:],
                        b2[:].unsqueeze(1).broadcast_to((128, XR, C)))

            # ================= Jacobi filter H on DVE ====================
            ev = _bc(e_col[:], (128, NB, C), axis=2)

            def jrow(i):
                return _bc(jc[:, i * C:(i + 1) * C], (128, NB, C))

            nc.gpsimd.tensor_copy(xs_a[:], jrow(0))                       # xs_m2
            nc.gpsimd.tensor_mul(htmp[:], xs_a[:], ev)
            nc.gpsimd.tensor_mul(htmp[:], htmp[:], jrow(2))
            nc.gpsimd.tensor_add(xs_b[:], htmp[:], jrow(1))               # xs_m1
            nc.gpsimd.tensor_add(hacc[:], xs_a[:], xs_b[:])
            xm2, xm1 = xs_a, xs_b
            for L in range(2, K + 1):
                r0 = 3 + 3 * (L - 2)
                nc.gpsimd.tensor_mul(htmp[:], xm1[:], ev)
                nc.gpsimd.tensor_mul(htmp[:], htmp[:], jrow(r0))
                nc.gpsimd.tensor_mul(htmp2[:], xm1[:], jrow(r0 + 1))
                nc.gpsimd.tensor_sub(htmp[:], htmp[:], htmp2[:])
                nc.gpsimd.tensor_mul(htmp2[:], xm2[:], jrow(r0 + 2))
                nc.gpsimd.tensor_sub(xm2[:], htmp[:], htmp2[:])           # nx
                nc.gpsimd.tensor_add(hacc[:], hacc[:], xm2[:])
                xm2, xm1 = xm1, xm2

            # ================= main loop over column blocks ==============
            cpeng = [
                lambda dst, src: nc.scalar.copy(dst, src),
                lambda dst, src: nc.vector.tensor_copy(dst, src),
                lambda dst, src: nc.gpsimd.tensor_copy(dst, src),
            ]
            with (
                tc.tile_pool(name="pmain", bufs=1, space="PSUM") as pm,
            ):
                pacc = pm.tile([128, 2048], F32, tag="pacc")       # 4 banks
                for b in range(NB):
                    # one contiguous DMA for the whole block's U (2MB bf16)
                    u_t = up.tile([128, RCH, 128], BF16, tag="u")
                    nc.sync.dma_start(
                        out=u_t[:],
                        in_=u_pr[b, :, :, :, :].rearrange("p g j c -> p (g j) c"),
                    )
                    ut_t = utp.tile([128, RCH, C + 128], BF16, tag="ut")
                    gp = sp.tile([128, 4, C], F32, tag="gp", bufs=2)
                    for rc in range(RCH):
                        q = rc % 2
                        if q == 0:
                            pt = pm.tile([128, 2, C + 128], F32, tag="pt", bufs=4)
                        # one matmul: [G contrib | transposed tile]
                        nc.tensor.matmul(
                            pt[:, q, :], lhsT=u_t[:, rc, :], rhs=zid[:, rc, :],
                            start=True, stop=True,
                        )
                        if q == 1:
                            dst = ut_t[:, rc - 1:rc + 1, :]
                            cpeng[(rc // 2) % 2](dst, pt[:])
                        if rc % 16 == 15:
                            g16 = rc // 16
                            # partial G over these 16 chunks
                            nc.vector.tensor_reduce(
                                out=gp[:, g16, :],
                                in_=ut_t[:, g16 * 16:(g16 + 1) * 16, 0:C]
                                .transpose([0, 2, 1]),
                                op=mybir.AluOpType.add, axis=mybir.AxisListType.X,
                            )
                            if g16 == 1:
                                nc.gpsimd.tensor_add(gp[:, 0, :], gp[:, 0, :],
                                                     gp[:, 1, :])
                            if g16 == 3:
                                nc.gpsimd.tensor_add(gp[:, 2, :], gp[:, 2, :],
                                                     gp[:, 3, :])
                    y_t = sp.tile([128, C], BF16, tag="y")
                    yf = sp.tile([128, C], F32, tag="yf")
                    nc.gpsimd.tensor_add(yf[:], gp[:, 0, :], gp[:, 2, :])
                    nc.gpsimd.tensor_mul(y_t[:], yf[:], hacc[:, b, :])
                    for rgp in range(16):
                        kq, off = rgp // 4, (rgp % 4) * 512
                        nc.tensor.matmul(
                            pacc[32 * kq:32 * kq + C, off:off + 512],
                            lhsT=y_t[:], rhs=ut_t[:, rgp * 4:(rgp + 1) * 4, C:C + 128],
                            start=(b == 0), stop=(b == NB - 1),
                            skip_group_check=True,
                            tile_position=(0, 32 * kq),
                        )
                # ---- emit out rows: transpose out^T quarters to [p, j, c] ----
                nc.scalar.copy(accsb[:], pacc[:])
                for rc in range(RCH):
                    kq, jj = rc // 16, rc % 16
                    pt2 = pm.tile([128, 2, C + 128], F32, tag="pt", bufs=4)
                    nc.tensor.transpose(
                        pt2[:, 0, 0:C],
                        accsb[32 * kq:32 * kq + C, jj * 128:(jj + 1) * 128],
                        id16x4[32 * kq:32 * kq + C, :],
                        tile_position=(32 * kq, 0),
                    )
                    cpeng[rc % 2](o_all[:, rc, :], pt2[:, 0, 0:C])
                nc.gpsimd.dma_start(out=rs_in[:], in_=o_all[:])
                nc.gpsimd.collective_compute(
                    "ReduceScatter", mybir.AluOpType.add, replica_groups=rg,
                    ins=[rs_in.opt()], outs=[rs_out.opt()],
                )

            # ============ log_softmax on the local (interleaved) shard ====
            # rs_out flat reinterpreted as [128, MYR, C] (same bytes)
            nc.sync.dma_start(
                out=smin[:],
                in_=rs_out[:].rearrange("q (a j) c -> (q a) j c", j=MYR))
            nc.vector.tensor_reduce(out=mneg[:, :, 0], in_=smin[:],
                                    op=mybir.AluOpType.max,
                                    axis=mybir.AxisListType.X, negate=True)
            nc.vector.tensor_add(sexp[:], smin[:],
                                 mneg[:].broadcast_to((128, MYR, C)))
            nc.scalar.activation(set_[:], sexp[:], mybir.ActivationFunctionType.Exp)
            nc.vector.tensor_reduce(out=ssum[:], in_=set_[:],
                                    op=mybir.AluOpType.add,
                                    axis=mybir.AxisListType.X)
            nc.scalar.activation(lns[:], ssum[:], mybir.ActivationFunctionType.Ln)
            nc.vector.tensor_sub(smout[:], sexp[:],
                                 lns[:].unsqueeze(2).broadcast_to((128, MYR, C)))
            nc.scalar.dma_start(
                out=out_sh[:].rearrange("p (j c) -> p j c", c=C), in_=smout[:])

    nc.compile()
    return nc


def _prep_inputs(origin_e, U, x, W1, b1, W2, b2, temp):
    import ml_dtypes

    def bf(a):
        return np.ascontiguousarray(np.asarray(a, np.float32)).astype(
            ml_dtypes.bfloat16)

    origin_e = np.ascontiguousarray(np.asarray(origin_e, np.float32))
    U = np.asarray(U, np.float32)
    x = np.asarray(x, np.float32)
    W1 = np.asarray(W1, np.float32)
    b1 = np.asarray(b1, np.float32)
    W2 = np.asarray(W2, np.float32)
    b2 = np.asarray(b2, np.float32)

    jc = _jacobi_coef_rows(temp)
    id128 = np.eye(128, dtype=np.float32)
    id16 = np.zeros((128, C), np.float32)
    for k in range(4):
        id16[32 * k:32 * k + C, :] = np.eye(C, dtype=np.float32)
    w1r = np.ascontiguousarray(
        W1.reshape(4, 128, HID).transpose(1, 0, 2).reshape(128, 4 * HID))
    shared = {
        "w1r": bf(w1r), "w2d": bf(W2),
        "b1c": np.ascontiguousarray(b1.reshape(HID, 1)),
        "b2r": np.ascontiguousarray(np.repeat(b2.reshape(1, C), 128, axis=0)),
        "jcd": jc, "id128d": bf(id128), "id16x4d": id16,
        "x_fullT": bf(x.T),
    }
    in_maps = []
    for i in range(NCORES):
        m = dict(shared)
        # u_prep[b, p, g, j, c] = U[1024g + 128j + p, 1024i + 128b + c]
        ush = U[:, i * SH:(i + 1) * SH]
        m["u_prep"] = np.ascontiguousarray(
            bf(ush).reshape(8, 8, 128, NB, 128).transpose(3, 2, 0, 1, 4))
        m["e_shard"] = np.ascontiguousarray(
            origin_e[i * SH:(i + 1) * SH].reshape(MYR, 128).T)
        in_maps.append(m)
    return in_maps


def _get_program():
    if "nc" not in _CACHE:
        _CACHE["nc"] = _build()
    return _CACHE["nc"]


def run(inputs, trace=False, **kw):
    nc = _get_program()
    in_maps = _prep_inputs(**inputs)
    res = run_bass_kernel_spmd(nc, in_maps, core_ids=list(range(NCORES)),
                               trace=trace, **kw)
    # out_shard core i flat f = q*1024 + j*16 + c -> out[128j + 16i + q, c]
    shards = np.stack([
        np.asarray(res.results[i]["out_shard"], np.float32)
        .reshape(C, RCH, C)
        for i in range(NCORES)
    ])  # [i, q, j, c]
    out = np.ascontiguousarray(
        shards.transpose(2, 0, 1, 3).reshape(N, C))  # row = 128j + 16i + q
    return out, res


def kernel(origin_e, U, x, W1, b1, W2, b2, temp):
    out, _ = run(dict(origin_e=origin_e, U=U, x=x, W1=W1, b1=b1, W2=W2,
                      b2=b2, temp=temp))
    return out
